# revision 9
# baseline (speedup 1.0000x reference)
"""Trainium2 Bass kernel for nn_DistanceNetwork (retrieval_knn).

out[b, s, j] = dot[s, j] / (||sup[s, b]|| * ||inp[b]|| + EPS)
  dot[s, j] = sum_d sup[s, j, d] * inp[j, d]

Sharding: S=8192 split across 8 cores (1024 each). Each core reads its
support slice + the replicated input_signal and writes its output slice
in [1024, B, B] s-major layout (contiguous 2 KiB bf16 rows per
partition, full DMA line rate); the host concatenates along s, upcasts
to f32 and transposes to the reference [B, S, B] layout.

Per 128-s tile (layout [128 part = s, free = (b d)]):
 - DVE: fused mul+cumsum custom op (DOT_SCAN) -> per-segment dot via
   strided cumsum differences; SQ_SCAN cumsum of squares for the first
   K_DVE b-segments; fast approximate reciprocal.
 - ACT: Square+accumulate (scaled by tnorm[b], folding the target norm
   into the accumulated sum) for the remaining b-segments; sqrt.
 - GpSimd: cumsum boundary diffs, the [B,B] outer-product broadcast
   multiply, and the SWDGE f32->bf16 cast stores.

EPS is dropped on-device: denom = norm*tnorm ~ O(100) for randn data,
so the reference's +1e-10 is ~1e-12 relative -- far below tolerance.
"""

import os
import sys

import numpy as np

for _p in ("/opt/trn_rl_repo", "/root/.axon_site/_ro/trn_rl_repo"):
    if os.path.isdir(_p) and _p not in sys.path:
        sys.path.insert(0, _p)

import concourse.bass as bass
import concourse.bacc as bacc
import concourse.mybir as mybir
from concourse.bass_utils import run_bass_kernel_spmd
from concourse.tile import TileContext

S, B, D = 8192, 32, 128
NCORES = 8
SL = S // NCORES          # 1024 s-rows per core
P = 128                   # partition tile of s
TILES = SL // P           # 8 s-tiles per core
BD = B * D                # 4096
F32 = mybir.dt.float32
BF16 = mybir.dt.bfloat16
X = mybir.AxisListType.X

# How many of the 32 b-segments' sum-of-squares DVE computes (via SQ_SCAN);
# the rest go to the Scalar engine as Square+accumulate chunks.
K_DVE = 22
KD = K_DVE * D


# --- custom DVE ops (registered at import; uop table is built per-NEFF) --- #

def _register_scan_ops():
    import concourse.dve_ops as dve_ops_mod
    from concourse.dve_ops import DveOp, OPS, CUSTOM_DVE_SPECS
    from concourse.dve_spec import Spec, Src0, Src1, AluOp, scan, sq, lower
    from concourse.dve_spec import _has_src1
    from concourse.dve_uop import DveOpSpec

    def reg(name, spec):
        if name in dve_ops_mod._SUB_OPCODE_FOR_NAME:
            return next(op for op in OPS if op.name == name)
        op = DveOp(name=name, spec=spec, subdim=False, uops_sha={})
        OPS.append(op)
        CUSTOM_DVE_SPECS[name] = spec
        row = dve_ops_mod._CUSTOM_DVE_ROW_BASE + len(OPS) - 1
        assert row < 0x20
        dve_ops_mod._SUB_OPCODE_FOR_NAME[name] = row
        for ver in ("v3", "v4"):
            try:
                spec_c = DveOpSpec(
                    name=name,
                    opcode=row,
                    uops=lower(spec, ver=ver),
                    rd1_en=_has_src1(spec),
                )
                op.uops_sha[ver] = spec_c.sha(ver)
            except Exception:
                pass
        return op

    dot_scan = reg(
        "ANTK_DOT_SCAN",
        Spec(
            body=scan(AluOp.ADD, Src0 * Src1),
            reference=lambda in0, in1, s0, s1, imm2: np.cumsum(
                in0.astype(np.float32) * in1.astype(np.float32), axis=-1
            ),
        ),
    )
    sq_scan = reg(
        "ANTK_SQ_SCAN",
        Spec(
            body=scan(AluOp.ADD, sq(Src0)),
            reference=lambda in0, in1, s0, s1, imm2: np.cumsum(
                np.square(in0.astype(np.float32)), axis=-1
            ),
        ),
    )
    return dot_scan, sq_scan


DOT_SCAN, SQ_SCAN = _register_scan_ops()


def _build_nc():
    nc = bacc.Bacc()
    sup = nc.declare_dram_parameter("support", [SL, B, D], F32, isOutput=False)
    inpr = nc.declare_dram_parameter("inp_rep", [P, BD], F32, isOutput=False)
    tnh = nc.declare_dram_parameter("tnorm", [1, B], F32, isOutput=False)
    tn2h = nc.declare_dram_parameter("tnorm2", [1, B], F32, isOutput=False)
    out = nc.declare_dram_parameter("out", [SL, B * B], BF16, isOutput=True)
    SQUARE = mybir.ActivationFunctionType.Square
    NCHUNK = 4
    CW = BD // NCHUNK     # 1024 columns per startup chunk

    with TileContext(nc) as tc:
        with (
            tc.tile_pool(name="const", bufs=1) as cpool,
            tc.tile_pool(name="sup", bufs=5) as suppool,
            tc.tile_pool(name="scan", bufs=2) as scpool,
            tc.tile_pool(name="outp", bufs=2) as opool,
            tc.tile_pool(name="small", bufs=3) as spool,
        ):
            # Startup: interleave inp_rep chunks (sync ring) with the first
            # sup tile's chunks (scalar ring) so the first quarter DOT_SCAN
            # fires after ~1 MB of DMA instead of a full replication chain.
            inp_rep = cpool.tile([P, BD], F32)
            tn_bc = cpool.tile([P, B], F32)
            tn2_bc = cpool.tile([P, B], F32)
            sup0 = suppool.tile([P, BD], F32, tag="sup")
            sup0_src = sup[0:P, :, :].rearrange("s b d -> s (b d)")
            with tc.high_priority():
                nc.scalar.dma_start(out=tn_bc[:], in_=tnh[:, :].broadcast_to([P, B]))
                nc.scalar.dma_start(
                    out=tn2_bc[:], in_=tn2h[:, :].broadcast_to([P, B])
                )
                for c in range(NCHUNK):
                    cs = slice(c * CW, (c + 1) * CW)
                    nc.sync.dma_start(out=inp_rep[:, cs], in_=inpr[:, cs])
                    nc.scalar.dma_start(out=sup0[:, cs], in_=sup0_src[:, cs])

            for t in range(TILES):
                if t == 0:
                    sup_t = sup0
                else:
                    sup_t = suppool.tile([P, BD], F32, tag="sup")
                    nc.sync.dma_start(
                        out=sup_t[:],
                        in_=sup[t * P:(t + 1) * P, :, :].rearrange(
                            "s b d -> s (b d)"
                        ),
                    )

                # sq2[p, b] = tnorm[b]^2 * sum_d sup^2: first K_DVE segments
                # via DVE cumsum + gpsimd boundary diffs (*tnorm^2), the rest
                # on ACT as Square+accumulate with scale=tnorm[b].
                sq2 = spool.tile([P, B], F32, tag="sq2")
                ssc = scpool.tile([P, KD + 1], F32, tag="sscan")
                nc.gpsimd.memset(ssc[:, 0:1], 0.0)
                nc.vector._custom_dve(
                    SQ_SCAN, out=ssc[:, 1:KD + 1], in0=sup_t[:, 0:KD]
                )
                sends = ssc[:, 1:KD + 1].rearrange("p (b d) -> p b d", d=D)
                sprevs = ssc[:, 0:KD].rearrange("p (b d) -> p b d", d=D)
                nc.gpsimd.tensor_sub(
                    sq2[:, 0:K_DVE],
                    sends[:, :, D - 1:D].squeeze(2),
                    sprevs[:, :, 0:1].squeeze(2),
                )
                nc.gpsimd.tensor_mul(
                    sq2[:, 0:K_DVE], sq2[:, 0:K_DVE], tn2_bc[:, 0:K_DVE]
                )
                scr = spool.tile([P, D], F32, tag="scr")
                for b in range(K_DVE, B):
                    nc.scalar.activation(
                        scr[:],
                        sup_t[:, b * D:(b + 1) * D],
                        SQUARE,
                        scale=tn_bc[:, b:b + 1],
                        accum_out=sq2[:, b:b + 1],
                    )

                # dot[p, j]: cumsum of sup*inp along (b d); per-segment sums
                # are differences of the padded cumsum at segment boundaries.
                dsc = scpool.tile([P, BD + 4], F32, tag="dscan")
                dot = spool.tile([P, B], F32, tag="dot")
                if t == 0:
                    # four quarter-scans, each firing as its input chunks land
                    H = CW
                    for h in range(NCHUNK):
                        base = h * (H + 1)
                        nc.gpsimd.memset(dsc[:, base:base + 1], 0.0)
                        nc.vector._custom_dve(
                            DOT_SCAN,
                            out=dsc[:, base + 1:base + 1 + H],
                            in0=sup_t[:, h * H:(h + 1) * H],
                            in1=inp_rep[:, h * H:(h + 1) * H],
                        )
                        hends = dsc[:, base + 1:base + 1 + H].rearrange(
                            "p (b d) -> p b d", d=D
                        )
                        hprevs = dsc[:, base:base + H].rearrange(
                            "p (b d) -> p b d", d=D
                        )
                        nc.gpsimd.tensor_sub(
                            dot[:, h * (B // 4):(h + 1) * (B // 4)],
                            hends[:, :, D - 1:D].squeeze(2),
                            hprevs[:, :, 0:1].squeeze(2),
                        )
                else:
                    nc.gpsimd.memset(dsc[:, 0:1], 0.0)
                    nc.vector._custom_dve(
                        DOT_SCAN, out=dsc[:, 1:BD + 1], in0=sup_t[:],
                        in1=inp_rep[:],
                    )
                    ends = dsc[:, 1:BD + 1].rearrange("p (b d) -> p b d", d=D)
                    prevs = dsc[:, 0:BD].rearrange("p (b d) -> p b d", d=D)
                    sub_eng = nc.vector if t == TILES - 1 else nc.gpsimd
                    sub_eng.tensor_sub(
                        dot[:],
                        ends[:, :, D - 1:D].squeeze(2),
                        prevs[:, :, 0:1].squeeze(2),
                    )

                # rden = 1 / sqrt(sq2)  (= 1/(norm*tnorm); eps dropped)
                sn = spool.tile([P, B], F32, tag="sn")
                nc.scalar.sqrt(sn[:], sq2[:])
                rden = spool.tile([P, B], F32, tag="rden")
                nc.vector.reciprocal_approx_fast(rden[:], sn[:])

                # outt[p, b, j] = rden[p, b] * dot[p, j] in fp32; the SWDGE
                # store casts f32 -> bf16. HBM layout is s-major: each
                # partition writes one contiguous 2 KiB row.
                outt = opool.tile([P, B * B], F32, tag="outt")
                out_dst = out[t * P:(t + 1) * P, :]
                if t == TILES - 1:
                    # tail: quarter the outer product on DVE and stagger four
                    # cast-stores so draining starts immediately
                    Q = B // 4
                    for q in range(4):
                        bs = slice(q * Q, (q + 1) * Q)
                        nc.vector.tensor_mul(
                            outt[:, q * Q * B:(q + 1) * Q * B].rearrange(
                                "p (b j) -> p b j", j=B
                            ),
                            rden[:, bs].unsqueeze(2).broadcast_to([P, Q, B]),
                            dot[:].unsqueeze(1).broadcast_to([P, Q, B]),
                        )
                        nc.gpsimd.dma_start(
                            out=out_dst[:, q * Q * B:(q + 1) * Q * B],
                            in_=outt[:, q * Q * B:(q + 1) * Q * B],
                        )
                else:
                    nc.gpsimd.tensor_mul(
                        outt[:].rearrange("p (b j) -> p b j", j=B),
                        rden[:].unsqueeze(2).broadcast_to([P, B, B]),
                        dot[:].unsqueeze(1).broadcast_to([P, B, B]),
                    )
                    # SWDGE queue drains in parallel with the sync-queue loads
                    nc.gpsimd.dma_start(out=out_dst, in_=outt[:])
    if not nc.is_finalized():
        nc.finalize()
    return nc


_NC = None
last_results = None


def _get_nc():
    global _NC
    if _NC is None:
        _NC = _build_nc()
    return _NC


def kernel(support_set: np.ndarray, input_signal: np.ndarray) -> np.ndarray:
    global last_results
    support_set = np.ascontiguousarray(support_set, dtype=np.float32)
    input_signal = np.ascontiguousarray(input_signal, dtype=np.float32)
    nc = _get_nc()
    tnorm = np.sqrt(np.sum(input_signal.astype(np.float32) ** 2, axis=1))
    tnorm = np.ascontiguousarray(tnorm.reshape(1, B), dtype=np.float32)
    tnorm2 = np.ascontiguousarray(tnorm * tnorm, dtype=np.float32)
    inp_rep = np.ascontiguousarray(
        np.broadcast_to(input_signal.reshape(1, BD), (P, BD)), dtype=np.float32
    )
    in_maps = [
        {
            "support": np.ascontiguousarray(support_set[i * SL:(i + 1) * SL]),
            "inp_rep": inp_rep,
            "tnorm": tnorm,
            "tnorm2": tnorm2,
        }
        for i in range(NCORES)
    ]
    res = run_bass_kernel_spmd(nc, in_maps, list(range(NCORES)))
    last_results = res
    # Each core returns [SL, B*B] bf16 (s-major). Concat along s, upcast,
    # and transpose to the reference [B, S, B] layout on the host.
    full = np.concatenate(
        [np.asarray(res.results[i]["out"]) for i in range(NCORES)], axis=0
    )
    full = full.astype(np.float32).reshape(S, B, B)
    return np.ascontiguousarray(full.transpose(1, 0, 2))
